# revision 59
# baseline (speedup 1.0000x reference)
"""Trainium2 Bass kernel for nn_CNNModel_82222853915196.

Model (per utterance x: (64, 512)):
  multiscale patch features (h in {8,16,32,64}) -> feats (8192,)
  out[t, :] = Wfc @ concat([x[:, t], feats]) + bfc

Factorization: feats is broadcast over t, so
  out = x.T @ Wfc1.T  +  broadcast(C),  C = Wfc2 @ feats + cconst
with Wfc1 = Wfc[:, :64], Wfc2 = Wfc[:, 64:], feature biases folded into
cconst on the host.

Key structure (all feature math fp16, fp32 PSUM accumulation):
 * j-pairing: x sits in a [128, 2048] tile whose lower 64 partitions hold
   x and upper 64 hold x shifted left one column (built with one on-device
   SBUF-SBUF DMA).  Masked per-offset patch weights for (j, j+1) stack
   into one K=128 stationary, halving the matmul count of every scale.
 * Wfc2 (6.55MB fp16, feature-permuted) loads once into SBUF via four
   12.8KB-per-partition-line DMAs split over both HWDGE rings; one
   PSUM-accumulated K=128 matmul per feature tile (64 total) consumes it.
 * h=32/h=64 features never round-trip through DRAM: h=32's PSUM layout is
   already partition-compatible with the C matmul; h=64 is fixed up with
   two PE transposes.  h=8/h=16 use a DRAM scatter/gather in a [kt, u, fp]
   layout whose runs are 64-256B (SWDGE-friendly).
 * The frames term is computed TRANSPOSED (out^T[o, t]) with Wfc1 as a
   reused stationary: 16 matmuls of N=512 staged straight into the fp16
   output tile.  After C arrives, in-place per-partition-column adds
   (spread over DVE/Pool/Act) finish the tile, and two [100 x 8KB-line]
   DMAs write it out.  The host transposes/upcasts.

Sharding: pure data parallel - 32 utterances -> 8 cores x 4. Weights
replicated; no cross-core communication (collectives cost ~70-100us in
cross-core skew under this runtime).
"""

import os
import sys
from contextlib import ExitStack

import numpy as np

for _p in ("/opt/trn_rl_repo", "/root/.axon_site/_ro/trn_rl_repo"):
    if os.path.isdir(_p) and _p not in sys.path:
        sys.path.insert(0, _p)

import concourse.bass as bass
import concourse.tile as tile
from concourse import bacc, mybir
from concourse.bass_utils import run_bass_kernel_spmd

NCORES = 8
NUTT = 4                 # utterances per core
T = 512
F = 64
OUT = 400
W = NUTT * T             # 2048, free width of the x tile
FP32 = mybir.dt.float32
FP16 = mybir.dt.float16
FP8 = mybir.dt.float8e4
NPF16 = np.float16

# wpack column offsets (fp16 [128, 4288]):
#   w8jp [128,128] | w16jp [128,512] | w32jp [128,2048] | wfc1t [64,400]
#   | cconst4 [4,400] | eye32 [32,32] | S8 [32,512] | S16 [64,256]
WP_W8, WP_W16, WP_W32 = 0, 128, 640
WP_FC1, WP_CC, WP_EYE = 2688, 3088, 3488
WP_S8, WP_S16 = 3520, 4032
WP_COLS = 4288


# ---------------------------------------------------------------------------
# host-side weight preparation
# ---------------------------------------------------------------------------

def _build_devindex():
    """devindex[kt, fp] = reference flat feature index m in [0, 8192)."""
    devindex = np.full((64, 128), -1, dtype=np.int64)
    # h=8: PSUM (q=k*4+o, u*64+p): kt = p//4, fp = (p%4)*32 + q
    for k in range(8):
        for p in range(64):
            for o in range(4):
                devindex[p // 4, (p % 4) * 32 + k * 4 + o] = (k * 64 + p) * 4 + o
    # h=16: (q=k*16+o, u*32+p): kt = 16 + p//2, fp = (p%2)*64 + q
    for k in range(4):
        for p in range(32):
            for o in range(16):
                devindex[16 + p // 2, (p % 2) * 64 + k * 16 + o] = \
                    2048 + (k * 32 + p) * 16 + o
    # h=32: (q=k*64+o, u*16+p): kt = 32 + p, fp = q  (partition-preserving)
    for k in range(2):
        for p in range(16):
            for o in range(64):
                devindex[32 + p, k * 64 + o] = 4096 + (k * 16 + p) * 64 + o
    # h=64 via PE transpose: kt = 48 + g*8 + p (g = o//128), fp = o%128
    for p in range(8):
        for o in range(256):
            devindex[48 + (o // 128) * 8 + p, o % 128] = 6144 + p * 256 + o
    assert devindex.min() >= 0
    return devindex


def _masked_paired(Wh, nk, h, no):
    """wp[r or 64+r, m*nk*no + k*no + o] = Wh[k, o, (r-k)*h + (2m or 2m+1)]."""
    w = np.zeros((64, h, nk * no), dtype=np.float32)
    for k in range(nk):
        for i in range(h):
            w[k + i, :, k * no:(k + 1) * no] = Wh[k].reshape(no, h, h)[:, i, :].T
    wp = np.zeros((128, (h // 2) * nk * no), dtype=np.float32)
    q = nk * no
    for m in range(h // 2):
        wp[0:64, m * q:(m + 1) * q] = w[:, 2 * m, :]
        wp[64:128, m * q:(m + 1) * q] = w[:, 2 * m + 1, :]
    return wp


def host_prep(W8, b8, W16, b16, W32, b32, W64, b64, Wfc, bfc):
    f32 = np.float32
    W8 = np.asarray(W8, f32); W16 = np.asarray(W16, f32)
    W32 = np.asarray(W32, f32); W64 = np.asarray(W64, f32)
    Wfc = np.asarray(Wfc, f32)
    b8 = np.asarray(b8, f32); b16 = np.asarray(b16, f32)
    b32 = np.asarray(b32, f32); b64 = np.asarray(b64, f32)
    bfc = np.asarray(bfc, f32)

    wpack = np.zeros((128, WP_COLS), dtype=f32)
    wpack[:, WP_W8:WP_W8 + 128] = _masked_paired(W8, 8, 8, 4)
    wpack[:, WP_W16:WP_W16 + 512] = _masked_paired(W16, 4, 16, 16)
    wpack[:, WP_W32:WP_W32 + 2048] = _masked_paired(W32, 2, 32, 64)
    wpack[0:64, WP_FC1:WP_FC1 + OUT] = Wfc[:, :64].T
    wpack[0:32, WP_EYE:WP_EYE + 32] = np.eye(32, dtype=f32)
    # partition-expansion (shift) stationaries.  The 1/64 scale compensates
    # the x64 scaling of the fp8 low Wfc2 block (keeps its values in e4m3's
    # normal range); both are exact powers of two.
    for pl in range(4):
        blk = wpack[0:32, WP_S8 + pl * 128:WP_S8 + (pl + 1) * 128]
        blk[:, pl * 32:(pl + 1) * 32] = np.eye(32, dtype=f32) / 64
    for pl in range(2):
        blk = wpack[0:64, WP_S16 + pl * 128:WP_S16 + (pl + 1) * 128]
        blk[:, pl * 64:(pl + 1) * 64] = np.eye(64, dtype=f32) / 64

    # w64wp[i, m*256+o] = W64[o, i*64+2m]; row 64+i holds j=2m+1
    w64 = W64.reshape(256, 64, 64)              # [o, i, j]
    w64wp = np.zeros((128, 32 * 256), dtype=f32)
    for m in range(32):
        w64wp[0:64, m * 256:(m + 1) * 256] = w64[:, :, 2 * m].T
        w64wp[64:128, m * 256:(m + 1) * 256] = w64[:, :, 2 * m + 1].T

    devindex = _build_devindex()
    Wfc2 = Wfc[:, 64:]
    perm = Wfc2[:, devindex.reshape(-1)].T      # [8192, 400], kt-major rows
    wfc2f = np.ascontiguousarray(
        perm.reshape(64, 128, OUT).transpose(1, 0, 2).reshape(128, 64 * OUT))

    fb = np.zeros(8192, dtype=np.float64)
    fb[0:2048] = np.broadcast_to(b8[:, None, :], (8, 64, 4)).reshape(-1)
    fb[2048:4096] = np.broadcast_to(b16[:, None, :], (4, 32, 16)).reshape(-1)
    fb[4096:6144] = np.broadcast_to(b32[:, None, :], (2, 16, 64)).reshape(-1)
    fb[6144:8192] = np.broadcast_to(b64[None, :], (8, 256)).reshape(-1)
    cconst = (Wfc2.astype(np.float64) @ fb + bfc.astype(np.float64)).astype(f32)
    wpack[0:NUTT, WP_CC:WP_CC + OUT] = np.tile(cconst.reshape(1, OUT), (NUTT, 1))

    import ml_dtypes
    np8 = mybir.dt.np(mybir.dt.float8e4)
    return {
        "wpack16": np.ascontiguousarray(wpack.astype(NPF16)),
        "w64wp": np.ascontiguousarray(w64wp.astype(NPF16)),
        "wfc2lo": np.ascontiguousarray((wfc2f[:, :32 * OUT] * 64.0).astype(np8)),
        "wfc2hi": np.ascontiguousarray(wfc2f[:, 32 * OUT:].astype(NPF16)),
    }


# ---------------------------------------------------------------------------
# device program
# ---------------------------------------------------------------------------

def build_program():
    nc = bacc.Bacc("TRN2", target_bir_lowering=False, debug=False)

    dram = dict(
        xw=nc.dram_tensor("xw", [128, W + WP_COLS], FP16, kind="ExternalInput"),
        w64wp=nc.dram_tensor("w64wp", [128, 8192], FP16, kind="ExternalInput"),
        wfc2lo=nc.dram_tensor("wfc2lo", [128, 32 * OUT], FP8, kind="ExternalInput"),
        wfc2hi=nc.dram_tensor("wfc2hi", [128, 32 * OUT], FP16, kind="ExternalInput"),
        out_t=nc.dram_tensor("out_t", [100, 16 * 512], FP16, kind="ExternalOutput"),
    )

    with tile.TileContext(nc) as tc:
        with ExitStack() as ctx:
            _emit(nc, tc, ctx, dram)

    nc.compile()
    return nc


def _emit(nc, tc, ctx, dram):
    scalar_dma = nc.scalar.dma_start
    gpsimd_dma = nc.gpsimd.dma_start
    sync_dma = nc.sync.dma_start

    const = ctx.enter_context(tc.tile_pool(name="const", bufs=1))
    stg = ctx.enter_context(tc.tile_pool(name="stg", bufs=2))
    ps = ctx.enter_context(tc.tile_pool(name="ps", bufs=1, space="PSUM"))
    psc = ctx.enter_context(tc.tile_pool(name="psc", bufs=1, space="PSUM"))
    psf = ctx.enter_context(tc.tile_pool(name="psf", bufs=4, space="PSUM"))
    pstp = ctx.enter_context(tc.tile_pool(name="pstp", bufs=1, space="PSUM"))

    # ---- loads (queue drain ~66ns/descriptor, one per partition, so only
    # four fat 128-descriptor DMAs): sync: xw (x + packed small weights),
    # wfc2hi; scalar: wfc2lo (fp8), w64wp.
    xwt = const.tile([128, W + WP_COLS], FP16, tag="xw")
    sync_dma(xwt[:], dram["xw"].ap())
    xw = xwt

    wfc2lo = const.tile([128, 32 * OUT], FP8, tag="wfc2lo")
    scalar_dma(wfc2lo[:], dram["wfc2lo"].ap())
    wfc2hi = const.tile([128, 32 * OUT], FP16, tag="wfc2hi")
    sync_dma(wfc2hi[:], dram["wfc2hi"].ap())
    w64wp = const.tile([128, 8192], FP16, tag="w64wp")
    scalar_dma(w64wp[:], dram["w64wp"].ap())

    # PE warmup: keep the tensor engine busy from the start so the DVFS
    # p-state ramps before the real matmul stream arrives.
    warm = const.tile([128, 512], FP16, tag="warm")
    nc.vector.memset(warm[:], 0.0)
    for _ in range(14):
        wps = psf.tile([128, 512], FP32, tag="framesps")
        nc.tensor.matmul(wps[:], warm[:, 0:128], warm[:], start=True, stop=True)

    w8jp = xw[:, W + WP_W8:W + WP_W8 + 128]
    w16jp = xw[:, W + WP_W16:W + WP_W16 + 512]
    w32jp = xw[:, W + WP_W32:W + WP_W32 + 2048]
    wfc1t = xw[0:64, W + WP_FC1:W + WP_FC1 + OUT]
    cconst4 = xw[0:NUTT, W + WP_CC:W + WP_CC + OUT]
    eye32 = xw[0:32, W + WP_EYE:W + WP_EYE + 32]
    s8 = xw[0:32, W + WP_S8:W + WP_S8 + 512]
    s16 = xw[0:64, W + WP_S16:W + WP_S16 + 256]

    feats8_16 = const.tile([128, 128], FP16, tag="feats8_16")
    f32t = const.tile([128, 64], FP16, tag="f32t")
    tp64sb = const.tile([128, 64], FP16, tag="tp64sb")
    ct_sb = const.tile([128, 16], FP32, tag="ct_sb")
    ct_sb16 = const.tile([128, 16], FP16, tag="ct_sb16")
    outstage = const.tile([128, 16 * 512], FP16, tag="outstage")

    cps = psc.tile([NUTT, OUT], FP32, tag="cps")

    f32t_r = f32t[:, :].rearrange("q (u p) -> q p u", u=NUTT)      # [128,16,4]
    tp64_r = tp64sb[:, :].rearrange("q (g u p) -> q g p u", g=2, u=NUTT)

    def cstat(kt):
        if kt < 16:
            return feats8_16[:, kt * NUTT:(kt + 1) * NUTT]
        if kt < 32:
            return feats8_16[:, 64 + (kt - 16) * NUTT: 64 + (kt - 15) * NUTT]
        if kt < 48:
            return f32t_r[:, kt - 32, :]
        g, p = (kt - 48) // 8, (kt - 48) % 8
        return tp64_r[:, g, p, :]

    def cmms(kts):
        for kt in kts:
            src_w = (wfc2lo[:, kt * OUT:(kt + 1) * OUT] if kt < 32 else
                     wfc2hi[:, (kt - 32) * OUT:(kt - 31) * OUT])
            nc.tensor.matmul(cps[:], cstat(kt), src_w,
                             start=(kt == 0), stop=(kt == 63))

    def frames(ots):
        """Transposed frames matmuls straight into the fp16 output tile."""
        for ot in ots:
            for u in range(NUTT):
                fps = psf.tile([100, 512], FP32, tag="framesps")
                nc.tensor.matmul(
                    fps[:], wfc1t[:, ot * 100:(ot + 1) * 100],
                    xw[0:64, u * T:(u + 1) * T], start=True, stop=True)
                idx = ot * NUTT + u
                dst = outstage[0:100, idx * 512:(idx + 1) * 512]
                if idx % 2:
                    nc.vector.tensor_copy(dst, fps[:])
                else:
                    nc.scalar.activation(dst, fps[:],
                                         mybir.ActivationFunctionType.Copy)

    # ---- scale h=8: 4 paired MMs K=128 M=32 N=256 -> PSUM (k*4+o, u*64+p)
    x8 = xw[:, 0:W].rearrange("i (u p j) -> i u p j", u=NUTT, j=8)
    acc = ps.tile([32, NUTT * 64], FP32, tag="fsmall")
    for m in range(4):
        nc.tensor.matmul(acc[:], w8jp[:, m * 32:(m + 1) * 32], x8[:, :, :, 2 * m],
                         start=(m == 0), stop=(m == 3))
    st = stg.tile([32, NUTT * 64], FP16, tag="f8st")
    nc.vector.tensor_copy(st[:], acc[:])
    # partition-expand on PE: feats8_16[pl*32+q, ph*4+u] = st[q, (u, ph, pl)]
    st8_r = st[:].rearrange("q (u ph pl) -> q ph u pl", u=NUTT, ph=16)
    f8x = pstp.tile([128, 64], FP32, tag="tpps")
    for pl in range(4):
        nc.tensor.matmul(f8x[:], s8[:, pl * 128:(pl + 1) * 128],
                         st8_r[:, :, :, pl], start=(pl == 0), stop=(pl == 3))
    nc.vector.tensor_copy(feats8_16[:, 0:64], f8x[:])

    # ---- scale h=16: 8 paired MMs K=128 M=64 N=128 -> PSUM (k*16+o, u*32+p)
    x16 = xw[:, 0:W].rearrange("i (u p j) -> i u p j", u=NUTT, j=16)
    acc = ps.tile([64, NUTT * 32], FP32, tag="fsmall")
    for m in range(8):
        nc.tensor.matmul(acc[:], w16jp[:, m * 64:(m + 1) * 64], x16[:, :, :, 2 * m],
                         start=(m == 0), stop=(m == 7))
    st = stg.tile([64, NUTT * 32], FP16, tag="f16st")
    nc.vector.tensor_copy(st[:], acc[:])
    # partition-expand: feats8_16[pl*64+q, 64 + ph*4+u] = st[q, (u, ph, pl)]
    st16_r = st[:].rearrange("q (u ph pl) -> q ph u pl", u=NUTT, ph=16)
    f16x = pstp.tile([128, 64], FP32, tag="tpps")
    for pl in range(2):
        nc.tensor.matmul(f16x[:], s16[:, pl * 128:(pl + 1) * 128],
                         st16_r[:, :, :, pl], start=(pl == 0), stop=(pl == 1))
    nc.vector.tensor_copy(feats8_16[:, 64:128], f16x[:])

    frames((0, 1))
    cmms(range(0, 16))
    frames((2, 3))

    # ---- scale h=32: 16 paired MMs K=128 M=128 N=64 -> PSUM (k*64+o, u*16+p)
    x32 = xw[:, 0:W].rearrange("i (u p j) -> i u p j", u=NUTT, j=32)
    acc = ps.tile([128, NUTT * 16], FP32, tag="fsmall")
    for m in range(16):
        nc.tensor.matmul(acc[:], w32jp[:, m * 128:(m + 1) * 128],
                         x32[:, :, :, 2 * m], start=(m == 0), stop=(m == 15))
    nc.vector.tensor_copy(f32t[:], acc[:])  # direct: fp = q, no DRAM trip

    cmms(range(16, 32))

    # ---- scale h=64: 32 paired MMs K=128 M=32 N=256 (x stationary, w moving)
    acc = ps.tile([NUTT * 8, 256], FP32, tag="fsmall")
    x64 = xw[:, 0:W].rearrange("i (u p j) -> i u p j", u=NUTT, j=64)
    for m in range(32):
        nc.tensor.matmul(acc[:], x64[:, :, :, 2 * m],
                         w64wp[:, m * 256:(m + 1) * 256],
                         start=(m == 0), stop=(m == 31))
    st64 = stg.tile([NUTT * 8, 256], FP16, tag="f64st")
    nc.vector.tensor_copy(st64[:], acc[:])
    # PE-transpose [32, 128]x2 -> [128, 32]: tp64sb[o%128, g*32 + u*8 + p]
    for g in range(2):
        tpp = pstp.tile([128, 32], FP16, tag="tpps")
        nc.tensor.transpose(tpp[:], st64[:, g * 128:(g + 1) * 128], eye32)
        nc.vector.tensor_copy(tp64sb[:, g * 32:(g + 1) * 32], tpp[:])

    cmms(range(32, 48))
    cmms(range(48, 64))

    # ---- C row: cps + cconst4, then PE-transpose to ct_sb[o%100, ot*4+u]
    csb16 = stg.tile([NUTT, OUT], FP16, tag="csb16")
    nc.vector.tensor_tensor(csb16[:], cps[:], cconst4, mybir.AluOpType.add)
    ctp = pstp.tile([128, 32], FP16, tag="tpps")
    for ot in range(4):
        nc.tensor.transpose(ctp[0:100, ot * 4:(ot + 1) * 4],
                            csb16[:, ot * 100:(ot + 1) * 100], eye32[0:4, 0:4])
    nc.vector.tensor_copy(ct_sb[0:100, :], ctp[0:100, 0:16])
    nc.vector.tensor_copy(ct_sb16[0:100, :], ctp[0:100, 0:16])

    # ---- finalize: outstage[p, idx*512+t] += C column (in place), 2 fat DMAs
    def ct_bcast(idx):
        base = ct_sb16[0:100, idx:idx + 1]
        return bass.AP(tensor=base.tensor, offset=base.offset,
                       ap=[[base.ap[0][0], 100], [0, 512]])

    for idx in range(16):
        dst = outstage[0:100, idx * 512:(idx + 1) * 512]
        if idx % 2 == 0:
            nc.scalar.activation(dst, dst,
                                 mybir.ActivationFunctionType.Identity,
                                 bias=ct_sb[0:100, idx:idx + 1], scale=1.0)
        else:
            eng = nc.vector if idx % 4 == 1 else nc.gpsimd
            eng.tensor_tensor(dst, dst, ct_bcast(idx), mybir.AluOpType.add)
        if idx == 7:
            for p, ring in ((0, sync_dma), (1, scalar_dma)):
                ring(bass.AP(tensor=dram["out_t"], offset=p * 50 * 16 * 512,
                             ap=[[16 * 512, 50], [1, 8 * 512]]),
                     outstage[p * 50:(p + 1) * 50, 0:8 * 512])
    for p, ring in ((0, sync_dma), (1, scalar_dma)):
        ring(bass.AP(tensor=dram["out_t"], offset=p * 50 * 16 * 512 + 8 * 512,
                     ap=[[16 * 512, 50], [1, 8 * 512]]),
             outstage[p * 50:(p + 1) * 50, 8 * 512:16 * 512])


_NC_CACHE = None


def _get_nc():
    global _NC_CACHE
    if _NC_CACHE is None:
        _NC_CACHE = build_program()
    return _NC_CACHE


# ---------------------------------------------------------------------------
# entry point
# ---------------------------------------------------------------------------

def run(inputs, trace=False, **kw):
    nc = _get_nc()
    prep = host_prep(inputs["W8"], inputs["b8"], inputs["W16"], inputs["b16"],
                     inputs["W32"], inputs["b32"], inputs["W64"], inputs["b64"],
                     inputs["Wfc"], inputs["bfc"])
    batch = np.asarray(inputs["batch"], np.float32)
    in_maps = []
    for c in range(NCORES):
        x4 = batch[NUTT * c:NUTT * (c + 1)].transpose(1, 0, 2).reshape(F, W)
        x4hp = np.zeros((128, W), dtype=NPF16)
        x4hp[0:64, :] = x4.astype(NPF16)
        x4hp[64:128, 0:W - 1] = x4[:, 1:].astype(NPF16)
        m = dict(prep)
        m["xw"] = np.ascontiguousarray(
            np.concatenate([x4hp, m.pop("wpack16")], axis=1))
        in_maps.append(m)
    res = run_bass_kernel_spmd(nc, in_maps, core_ids=list(range(NCORES)),
                               trace=trace, **kw)
    outs = []
    for r in res.results:
        o = r["out_t"].astype(np.float32)          # [100, 16*512]
        o = o.reshape(100, 4, NUTT, 512)           # [p, ot, u, t]
        outs.append(o.transpose(2, 3, 1, 0).reshape(W, OUT))
    return np.concatenate(outs, axis=0), res


def kernel(**inputs):
    out, _ = run(inputs)
    return out
